# revision 17
# baseline (speedup 1.0000x reference)
"""Trainium2 Bass kernel for nn_CustomMultiheadAttention_85143431676311.

Reference computation (S=2048, B=2, E=1024, H=16, Dh=64):
    Q = query @ Wq.T + bq ; K, V likewise     (split into H heads of Dh)
    scores = Q K^T / sqrt(Dh) + attn_bias
    attn = softmax(scores, axis=-1)           # (H,B,S,S) -- full output
    out  = diag(attn) * V                     # einsum 'nbll,nbld->nbld'
    y    = out @ Wo.T + bo                    # (S,B,E)
    returns (y, attn)

Sharding: 8 cores, head-parallel. Core c owns heads {2c, 2c+1} for both
batches (Megatron column-parallel QKV, row-parallel Wo; partial y summed on
host). attn_bias and the (transposed) activations are replicated.

Per-core layouts are "transposed": positions are ordered b-major
(r = b*2048 + s), head-dims live on SBUF partitions.
"""

import os
import numpy as np

import concourse.bass as bass
import concourse.mybir as mybir
import concourse.tile as tile
from concourse import bacc
from concourse.bass_utils import run_bass_kernel_spmd
from concourse.masks import make_identity
from concourse.vector_clock import ScopedClock

S = 2048
B = 2
E = 1024
H = 16
DH = 64
P = 128
R = S * B           # 4096 positions, b-major
N_CORES = 8
HPC = H // N_CORES  # 2 heads per core
KO = E // P         # 8 contraction chunks for the projections
QR = S // P         # 16 q-row strips
F32 = mybir.dt.float32
AF = mybir.ActivationFunctionType
ALU = mybir.AluOpType

LAST_RESULT = None  # stash of BassKernelResults for test harnesses


class _TC(tile.TileContext):
    """TileContext with the end-of-kernel drain's sem waits split across
    single-wait SP NOPs (walrus here rejects >1 sync wait per instruction)."""

    def _drain_and_barrier(self, tick_clock, wait_clock):
        nc = self.nc
        drain_inst = nc.sync.drain()
        wait_clock.add_sem_waits(
            drain_inst.ins, ScopedClock({None: tick_clock.global_clock})
        )
        si = drain_inst.ins.sync_info
        if si is not None and len(si.on_wait) > 1:
            waits = list(si.on_wait)
            drain_inst.ins.sync_info = mybir.SyncInfo(
                on_wait=[waits[0]], on_update=list(si.on_update)
            )
            for w in waits[1:]:
                n = nc.sync.nop(nofuse=True)
                n.ins.sync_info = mybir.SyncInfo(on_wait=[w], on_update=[])

        nc.all_engine_barrier()
        assert self.sems is not None
        popped = nc._tile_sem_poison_stack.pop()
        assert popped is self._sem_poison
        nc.clear_and_free_semaphores(list(self.sems.allocated().values()))
        nc.all_engine_barrier()


def _build():
    nc = bacc.Bacc(trn_type="TRN2")

    qT = nc.dram_tensor("qT", (E, R), F32, kind="ExternalInput")
    kT = nc.dram_tensor("kT", (E, R), F32, kind="ExternalInput")
    vT = nc.dram_tensor("vT", (E, R), F32, kind="ExternalInput")
    biasmat = nc.dram_tensor("biasmat", (S, S), F32, kind="ExternalInput")
    wqT = nc.dram_tensor("wqT", (E, P), F32, kind="ExternalInput")
    wkT = nc.dram_tensor("wkT", (E, P), F32, kind="ExternalInput")
    wvT = nc.dram_tensor("wvT", (E, P), F32, kind="ExternalInput")
    woT = nc.dram_tensor("woT", (P, E), F32, kind="ExternalInput")
    bqv = nc.dram_tensor("bqv", (P, 1), F32, kind="ExternalInput")
    bkv = nc.dram_tensor("bkv", (P, 1), F32, kind="ExternalInput")
    bvv = nc.dram_tensor("bvv", (P, 1), F32, kind="ExternalInput")
    attn_o = nc.dram_tensor("attn_o", (HPC * B, S, S), F32, kind="ExternalOutput")
    y_o = nc.dram_tensor("y_o", (R, E), F32, kind="ExternalOutput")
    debug = bool(int(os.environ.get("KERNEL_DEBUG", "0")))
    if debug:
        dflat_o = nc.dram_tensor("dflat_o", (B, P, S), F32, kind="ExternalOutput")
        outT_o = nc.dram_tensor("outT_o", (B, P, S), F32, kind="ExternalOutput")

    with _TC(nc) as tc:
        with (
            tc.tile_pool(name="const", bufs=1) as const,
            tc.tile_pool(name="acts", bufs=1) as acts,
            tc.tile_pool(name="xs", bufs=3) as xs,
            tc.tile_pool(name="bia", bufs=2) as bia,
            tc.tile_pool(name="att", bufs=3) as att,
            tc.tile_pool(name="sm", bufs=6) as sm,
            tc.tile_pool(name="scr", bufs=2) as scr,
            tc.tile_pool(name="dfl", bufs=1) as dfl,
            tc.tile_pool(name="outp", bufs=1) as outp,
            tc.tile_pool(name="ys", bufs=3) as ys,
            tc.tile_pool(name="ps", bufs=2, space="PSUM") as ps,
        ):
            ident = const.tile([P, P], F32, tag="ident")
            make_identity(nc, ident)
            # ones_h[h][:, h*64:(h+1)*64] == 1, else 0: column-sum matmuls with
            # these as stationary land the result on that head's partitions.
            ones_h = []
            for h_loc in range(HPC):
                t = const.tile([P, P], F32, tag=f"ones_h{h_loc}", name=f"ones_h{h_loc}")
                nc.gpsimd.memset(t, 0.0)
                nc.gpsimd.memset(t[:, h_loc * DH : (h_loc + 1) * DH], 1.0)
                ones_h.append(t)

            w_sb = {}
            for nm, dram in (("wq", wqT), ("wk", wkT), ("wv", wvT)):
                t = const.tile([P, KO, P], F32, tag=nm, name=nm)
                nc.sync.dma_start(t[:], dram.rearrange("(ko p) m -> p ko m", p=P))
                w_sb[nm] = t
            woT_sb = const.tile([P, E], F32, tag="wo")
            nc.sync.dma_start(woT_sb[:], woT[:, :])
            b_sb = {}
            for nm, dram in (("bq", bqv), ("bk", bkv), ("bv", bvv)):
                t = const.tile([P, 1], F32, tag=nm, name=nm)
                nc.sync.dma_start(t[:], dram[:, :])
                b_sb[nm] = t

            # ---- projections: {q,k,v}T_sb = (W_slice @ X^T) [+ bias, Q/8] ----
            qT_sb = acts.tile([P, R], F32, tag="qT")
            kT_sb = acts.tile([P, R], F32, tag="kT")
            vT_sb = acts.tile([P, R], F32, tag="vT")
            HALF = R // 2
            for xdram, w, bvec, scale, dest in (
                (qT, w_sb["wq"], b_sb["bq"], 0.125, qT_sb),
                (kT, w_sb["wk"], b_sb["bk"], 1.0, kT_sb),
                (vT, w_sb["wv"], b_sb["bv"], 1.0, vT_sb),
            ):
                for half in range(2):
                    pst = ps.tile([P, HALF], F32, tag="ps")
                    for ko in range(KO):
                        xt = xs.tile([P, HALF], F32, tag="xt")
                        nc.sync.dma_start(
                            xt[:],
                            xdram[ko * P : (ko + 1) * P, half * HALF : (half + 1) * HALF],
                        )
                        for j in range(HALF // 512):
                            nc.tensor.matmul(
                                pst[:, j * 512 : (j + 1) * 512],
                                lhsT=w[:, ko, :],
                                rhs=xt[:, j * 512 : (j + 1) * 512],
                                start=(ko == 0),
                                stop=(ko == KO - 1),
                            )
                    nc.scalar.activation(
                        dest[:, half * HALF : (half + 1) * HALF],
                        pst[:],
                        AF.Identity,
                        bias=b_sb_ap(bvec),
                        scale=scale,
                    )

            # ---- attention: scores -> softmax -> attn out + diagonal ----
            # dstack[b] rows h_loc*64..+64 hold attn[h,b,q,q] replicated x64
            dstack = [
                dfl.tile([P, S], F32, tag=f"dstack{b}", name=f"dstack{b}")
                for b in range(B)
            ]
            for qr in range(QR):
                bias_t = bia.tile([P, S], F32, tag="bias")
                nc.sync.dma_start(bias_t[:], biasmat[qr * P : (qr + 1) * P, :])
                for hb in range(HPC * B):
                    h_loc, b = divmod(hb, B)
                    hsl = slice(h_loc * DH, (h_loc + 1) * DH)
                    spst = ps.tile([P, S], F32, tag="ps")
                    for j in range(S // 512):
                        js = slice(j * 512, (j + 1) * 512)
                        nc.tensor.matmul(
                            spst[:, js], lhsT=ident[:], rhs=bias_t[:, js],
                            start=True, stop=False,
                        )
                        nc.tensor.matmul(
                            spst[:, js],
                            lhsT=qT_sb[hsl, b * S + qr * P : b * S + (qr + 1) * P],
                            rhs=kT_sb[hsl, b * S + j * 512 : b * S + (j + 1) * 512],
                            start=False, stop=True,
                        )
                    e_t = att.tile([P, S], F32, tag="et")
                    rs = sm.tile([P, 1], F32, tag="rs")
                    nc.scalar.activation(
                        e_t[:], spst[:], AF.Exp, bias=0.0, scale=1.0, accum_out=rs[:]
                    )
                    rcp = sm.tile([P, 1], F32, tag="rcp")
                    nc.vector.reciprocal(rcp[:], rs[:])
                    nc.vector.tensor_scalar_mul(e_t[:], e_t[:], rcp[:])
                    # diagonal of this strip: mask with identity, column-sum via PE
                    msk = scr.tile([P, P], F32, tag="msk")
                    nc.vector.tensor_tensor(
                        msk[:], e_t[:, qr * P : (qr + 1) * P], ident[:], ALU.mult
                    )
                    # scores PSUM is dead now; reuse its first bank: the masked
                    # ones stationary puts colsum(msk)=diag on this head's rows
                    dps = spst[:, 0:P]
                    nc.tensor.matmul(
                        dps, lhsT=ones_h[h_loc][:], rhs=msk[:], start=True, stop=True
                    )
                    hrows = slice(h_loc * DH, (h_loc + 1) * DH)
                    nc.vector.tensor_copy(
                        dstack[b][hrows, qr * P : (qr + 1) * P], dps[hrows, :]
                    )
                    nc.sync.dma_start(attn_o[hb, qr * P : (qr + 1) * P, :], e_t[:])

            # ---- out = diag * V ; y_part = out^T.T @ WoT_slice ----
            for b in range(B):
                outT = outp.tile([P, S], F32, tag=f"outT{b}", name=f"outT{b}")
                nc.vector.tensor_tensor(
                    outT[:], vT_sb[:, b * S : (b + 1) * S], dstack[b][:], ALU.mult
                )
                if debug:
                    nc.sync.dma_start(outT_o[b], outT[:])
                    nc.sync.dma_start(dflat_o[b], dstack[b][:])
                for rt in range(S // P):
                    yps = ps.tile([P, E], F32, tag="ps")
                    for j in range(E // 512):
                        nc.tensor.matmul(
                            yps[:, j * 512 : (j + 1) * 512],
                            lhsT=outT[:, rt * P : (rt + 1) * P],
                            rhs=woT_sb[:, j * 512 : (j + 1) * 512],
                            start=True, stop=True,
                        )
                    ysb = ys.tile([P, E], F32, tag="ysb")
                    if rt % 2 == 0:
                        nc.vector.tensor_copy(ysb[:], yps[:])
                    else:
                        nc.scalar.copy(ysb[:], yps[:])
                    nc.sync.dma_start(
                        y_o[b * S + rt * P : b * S + (rt + 1) * P, :], ysb[:]
                    )
    nc.finalize()
    return nc


def b_sb_ap(t):
    return t[:, 0:1]


def prepare_in_maps(query, key, value, attn_bias, Wq, bq, Wk, bk, Wv, bv, Wo, bo):
    query = np.asarray(query, dtype=np.float32)
    key = np.asarray(key, dtype=np.float32)
    value = np.asarray(value, dtype=np.float32)
    attn_bias = np.ascontiguousarray(np.asarray(attn_bias, dtype=np.float32))
    Wq = np.asarray(Wq, dtype=np.float32)
    Wk = np.asarray(Wk, dtype=np.float32)
    Wv = np.asarray(Wv, dtype=np.float32)
    Wo = np.asarray(Wo, dtype=np.float32)
    bq = np.asarray(bq, dtype=np.float32)
    bk = np.asarray(bk, dtype=np.float32)
    bv = np.asarray(bv, dtype=np.float32)
    bo = np.asarray(bo, dtype=np.float32)

    # (S,B,E) -> (E, B*S) with positions b-major
    qT = np.ascontiguousarray(query.transpose(2, 1, 0).reshape(E, R))
    kT = np.ascontiguousarray(key.transpose(2, 1, 0).reshape(E, R))
    vT = np.ascontiguousarray(value.transpose(2, 1, 0).reshape(E, R))
    WqT = Wq.T
    WkT = Wk.T
    WvT = Wv.T
    WoT = np.ascontiguousarray(Wo.T)

    in_maps = []
    for c in range(N_CORES):
        sl = slice(c * P, (c + 1) * P)
        in_maps.append(
            {
                "qT": qT,
                "kT": kT,
                "vT": vT,
                "biasmat": attn_bias,
                "wqT": np.ascontiguousarray(WqT[:, sl]),
                "wkT": np.ascontiguousarray(WkT[:, sl]),
                "wvT": np.ascontiguousarray(WvT[:, sl]),
                "woT": np.ascontiguousarray(WoT[sl, :]),
                "bqv": np.ascontiguousarray((bq[sl] * 0.125).reshape(P, 1)),
                "bkv": np.ascontiguousarray(bk[sl].reshape(P, 1)),
                "bvv": np.ascontiguousarray(bv[sl].reshape(P, 1)),
            }
        )
    return in_maps


def kernel(query, key, value, attn_bias, Wq, bq, Wk, bk, Wv, bv, Wo, bo):
    global LAST_RESULT
    bo = np.asarray(bo, dtype=np.float32)
    in_maps = prepare_in_maps(
        query, key, value, attn_bias, Wq, bq, Wk, bk, Wv, bv, Wo, bo
    )

    nc = _build()
    trace = bool(int(os.environ.get("KERNEL_TRACE", "0")))
    res = run_bass_kernel_spmd(
        nc, in_maps, core_ids=list(range(N_CORES)), trace=trace
    )
    LAST_RESULT = res

    attn = np.concatenate([r["attn_o"] for r in res.results], axis=0)
    attn = attn.reshape(H, B, S, S)
    y = np.zeros((R, E), dtype=np.float32)
    for r in res.results:
        y += r["y_o"]
    y += bo[None, :]
    out = y.reshape(B, S, E).transpose(1, 0, 2)
    return np.ascontiguousarray(out), attn
